# revision 52
# baseline (speedup 1.0000x reference)
"""Trainium2 Bass kernel for nn_Block_13950053777949 (dense transformer block).

Strategy: data-parallel over batch (B=8 == 8 NeuronCores), zero collectives.
Each core processes one batch element x[b] = [T=2048, C=384] in TRANSPOSED
layout [C partitions, T free].

Key structure (ScalarE-bound pipeline, ~364us on healthy HW):
  - The hard floor is ScalarE: exp over all T*T*H attention logits
    (~25M elems/core ~= 200us at 1 elem/lane/cycle @1.2GHz). Everything
    else is organized to hide under that exp stream.
  - Score matmuls (contraction = head_size = 64) run as row-packed
    tile_position pairs: head 2p on PE rows 0-63, head 2p+1 on rows
    64-127, writing the two halves of one [128,1024] PSUM tile
    (different banks) concurrently. One [128,1024] exp (fp8 out) per
    (pair, chunk, s-tile).
  - AV runs in fp8 DoubleRow (virtual K=256, two s-tiles per matmul,
    halving the AV instruction stream). Stationary is [ones*64 | V_h]:
    out rows 0:64 = softmax denominator replicated (base partition 0
    for reciprocal_approx_fast), rows 64:128 = attn out. fp8 rounding
    of probabilities/V averages out over the 2048-term contraction.
  - Tokens are independent after attention: proj + LN2 + MLP are
    emitted as PIECES injected between the next chunk's attention
    s-tile groups, so every engine FIFO alternates exp-feeding matmuls
    with bites of MLP work. LN1+QKV are likewise interleaved into the
    first attention pair; big weight DMAs are deferred behind the x
    chunks on the serial sync DMA queue.
  - GELU(tanh-approx) ~= 0.5*u*(1+tanh(0.851u)) (== u*sigmoid(1.702u),
    ~2.4e-3 relative effect): one ScalarE Tanh + one fused
    scalar_tensor_tensor on VectorE, 0.5 folded into W2 host-side.
    Tanh/Exp/Sqrt keep ScalarE in two table sets total (sqrt at
    startup only); LN rsqrt = ACT Sqrt + reciprocal_approx_fast (LN1)
    or a VectorE Newton chain (LN2) — no Ln anywhere.
  - Weight matmuls bf16 with fp32 PSUM accumulation; LN stats via
    all-ones matmuls.
"""

import math
import numpy as np
import ml_dtypes

B, T, C = 8, 2048, 384
H, HS = 6, 64
NP = H // 2            # 3 head pairs
CT = C // 128          # 3 c-tiles
NST = T // 128         # 16 s-tiles
NCH = T // 512         # 4 token chunks
C4 = 4 * C             # 1536
JT = C4 // 128         # 12 j-tiles
EPS = 1e-5
GELU_B = 0.851    # gelu(u) ~= 0.5*u*(1+tanh(0.851*u)) = u*sigmoid(1.702*u);
                  # the 0.5 is folded into W2 host-side.

_BF = ml_dtypes.bfloat16


def build_program(repeat=1):
    from contextlib import ExitStack
    import concourse.bacc as bacc
    import concourse.tile as tile
    import concourse.mybir as mybir

    f32 = mybir.dt.float32
    bf = mybir.dt.bfloat16
    f8 = mybir.dt.float8e4
    AF = mybir.ActivationFunctionType
    PM = mybir.MatmulPerfMode

    nc = bacc.Bacc("TRN2", debug=False, enable_asserts=False)

    d_xb = nc.dram_tensor("xb", [C, T], bf, kind="ExternalInput").ap()
    d_xbo = nc.dram_tensor("xbo", [C, T], f32, kind="ExternalInput").ap()
    d_wq = nc.dram_tensor("wq", [C, C], bf, kind="ExternalInput").ap()
    d_wk = nc.dram_tensor("wk", [C, C], bf, kind="ExternalInput").ap()
    d_wv = nc.dram_tensor("wv", [C, C], bf, kind="ExternalInput").ap()
    d_wo = nc.dram_tensor("wo", [128, NP, C], bf, kind="ExternalInput").ap()
    d_w1 = nc.dram_tensor("w1", [C, C4], bf, kind="ExternalInput").ap()
    d_w2 = nc.dram_tensor("w2", [C4, C], bf, kind="ExternalInput").ap()
    d_cones = nc.dram_tensor("cones", [128, 128], bf, kind="ExternalInput").ap()
    d_out = nc.dram_tensor("out", [C, T], f32, kind="ExternalOutput").ap()

    with tile.TileContext(nc) as tc, ExitStack() as top:
        # ------------- persistent pool (weights/constants) -------------
        pw = top.enter_context(tc.tile_pool(name="pw", bufs=1))
        wq_sb = pw.tile([128, CT, C], bf, name="wq_sb", tag="wq_sb")
        wk_sb = pw.tile([128, CT, C], bf, name="wk_sb", tag="wk_sb")
        wv_sb = pw.tile([128, CT, C], bf, name="wv_sb", tag="wv_sb")
        # qkv weight DMAs are emitted after chunk-0's x DMAs (below) so the
        # LN1 critical chain starts as early as possible.
        wo_sb = pw.tile([128, NP, C], bf, name="wo_sb", tag="wo_sb")
        w1_sb = pw.tile([128, CT, C4], bf, name="w1_sb", tag="w1_sb")
        w2_sb = pw.tile([128, JT, C], bf, name="w2_sb", tag="w2_sb")
        # DMAs for these are emitted after the first attention pair so the
        # x chunks reach the head of the (serial) sync DMA queue first.
        cones = pw.tile([128, 128], bf, name="cones", tag="cones")
        nc.sync.dma_start(cones, d_cones)
        zcol = pw.tile([128, 1], f32, name="zcol", tag="zcol")
        nc.vector.memset(zcol, 0.0)
        epscol = pw.tile([128, 1], f32, name="epscol", tag="epscol")
        nc.vector.memset(epscol, EPS)

        ALU = mybir.AluOpType

        def rsqrt_dve(pool, var_ps, pfx):
            """rr = rsqrt(var_ps + EPS) on VectorE (deg-2 init + 1 Newton).
            Avoids ACT Ln so the whole kernel stays in one table set."""
            # y1 variance is ~1 +/- 0.15, so a linear init y0 = 1 - d/2
            # (err <= 3/8 d^2 ~ 0.01) converges in one Newton step (~2e-4):
            # two fewer serial VectorE ops on the tail-critical chain.
            y0 = pool.tile([128, 512], f32, name=f"{pfx}y0", tag="rsq_y0",
                           bufs=1)
            nc.vector.tensor_scalar(y0, var_ps, -0.5, 1.5 - EPS / 2,
                                    ALU.mult, ALU.add)
            t = pool.tile([128, 512], f32, name=f"{pfx}t", tag="rsq_t",
                          bufs=1)
            nc.vector.tensor_mul(t, y0, y0)
            t2 = pool.tile([128, 512], f32, name=f"{pfx}t2", tag="rsq_t2",
                           bufs=1)
            nc.vector.scalar_tensor_tensor(t2, var_ps, EPS, t,
                                           ALU.add, ALU.mult)
            nc.vector.tensor_scalar(t2, t2, -0.5, 1.5, ALU.mult, ALU.add)
            rr = pool.tile([128, 512], f32, name=f"{pfx}rr", tag="rsq_rr")
            nc.vector.tensor_mul(rr, y0, t2)
            return rr

        for _rep in range(repeat):
          with ExitStack() as reps:
            # per-rep long-lived tiles
            p_pers = reps.enter_context(
                tc.tile_pool(name=f"pers{_rep}", bufs=1))
            hh = [p_pers.tile([128, T], bf, name=f"hh{i}", tag=f"hh{i}")
                  for i in range(CT)]
            q_sb = [p_pers.tile([128, T], bf, name=f"q{p}", tag=f"q{p}")
                    for p in range(NP)]
            k_sb = [p_pers.tile([128, T], bf, name=f"k{p}", tag=f"k{p}")
                    for p in range(NP)]
            # AV stationary in fp8 DoubleRow layout [s-tile-pair, head,
            # pair-parity, m]: cols 0:64 = ones (denominator -> PSUM rows
            # 0:64, base 0 for reciprocal_approx_fast), cols 64:128 = V.
            # DoubleRow contracts two s-tiles (virtual K=256) per matmul,
            # halving the AV instruction stream on TensorE.
            vaug = p_pers.tile([128, NST // 2, H, 2, 128], f8, name="vaug",
                               tag="vaug")
            nc.gpsimd.memset(vaug[:, :, :, :, 0:HS], 1.0)

            # shared PSUM work pool ([128,512] fp32 tiles, 2 banks)
            ps_w = reps.enter_context(
                tc.tile_pool(name=f"ps_w{_rep}", bufs=2, space="PSUM"))

            # ============ LN1 + QKV, emitted per 512-chunk ============
            # K/Q/V for chunk c need only hh[:, chunk c]; ln1_chunk(c) is
            # emitted interleaved with the first attention pair's s-tile
            # groups so the exp stream starts as soon as chunk 0 is done.
            p_ln1 = reps.enter_context(tc.tile_pool(name=f"p_ln1{_rep}",
                                                    bufs=2))

            xb0 = []
            for kt in range(CT):
                t = p_ln1.tile([128, 512], bf, name=f"xb{kt}",
                               tag=f"xb{kt}")
                nc.sync.dma_start(t, d_xb[128 * kt:128 * (kt + 1), 0:512])
                xb0.append(t)
            if _rep == 0:
                nc.sync.dma_start(
                    wk_sb, d_wk.rearrange("(kt p) m -> p kt m", p=128))
                nc.sync.dma_start(
                    wq_sb, d_wq.rearrange("(kt p) m -> p kt m", p=128))
                nc.sync.dma_start(
                    wv_sb, d_wv.rearrange("(kt p) m -> p kt m", p=128))

            def ln1_chunk(c):
                    sl = slice(512 * c, 512 * (c + 1))
                    if c == 0:
                        xbt = xb0
                    else:
                        xbt = []
                        for kt in range(CT):
                            t = p_ln1.tile([128, 512], bf, name=f"xb{kt}",
                                           tag=f"xb{kt}")
                            nc.sync.dma_start(
                                t, d_xb[128 * kt:128 * (kt + 1), sl])
                            xbt.append(t)
                    mu = ps_w.tile([128, 512], f32, name="mu", tag="wps")
                    for kt in range(CT):
                        nc.tensor.matmul(mu, cones, xbt[kt],
                                         start=(kt == 0), stop=(kt == CT - 1))
                    xc = []
                    for kt in range(CT):
                        t = p_ln1.tile([128, 512], bf, name=f"xc{kt}",
                                       tag=f"xc{kt}")
                        nc.vector.tensor_sub(t, xbt[kt], mu)
                        xc.append(t)
                    sq = []
                    for kt in range(CT):
                        t = p_ln1.tile([128, 512], bf, name=f"sq{kt}",
                                       tag=f"sq{kt}", bufs=1)
                        nc.vector.tensor_mul(t, xc[kt], xc[kt])
                        sq.append(t)
                    var = ps_w.tile([128, 512], f32, name="var", tag="wps")
                    for kt in range(CT):
                        nc.tensor.matmul(var, cones, sq[kt],
                                         start=(kt == 0), stop=(kt == CT - 1))
                    # LN1 rsqrt = approx_recip(sqrt(var+eps)). The ACT Sqrt
                    # ops all run before the first attention Exp in the ACT
                    # FIFO, so the sqrt table set loads once at startup and
                    # the exp set once after — no mid-kernel table thrash.
                    sd = p_ln1.tile([128, 512], f32, name="sd", tag="sd")
                    nc.scalar.activation(sd, var, AF.Sqrt, bias=epscol)
                    rr = p_ln1.tile([128, 512], f32, name="rr", tag="rr")
                    nc.vector.reciprocal_approx_fast(rr, sd)
                    for kt in range(CT):
                        nc.vector.tensor_mul(hh[kt][:, sl], xc[kt], rr)

                    # K/Q for pair 0 only (pairs 1/2 are emitted inside
                    # their attention pairs' pre_groups). PSUM->SBUF copies
                    # go through the (startup-idle) ScalarE to keep VectorE
                    # off the critical path.
                    for p in (0,):
                        for (wsb, dst) in ((wk_sb, k_sb), (wq_sb, q_sb)):
                            ps = ps_w.tile([128, 512], f32, name="qk_ps",
                                           tag="wps")
                            for kt in range(CT):
                                nc.tensor.matmul(
                                    ps,
                                    wsb[:, kt, 128 * p:128 * (p + 1)],
                                    hh[kt][:, sl],
                                    start=(kt == 0), stop=(kt == CT - 1))
                            nc.scalar.activation(dst[p][:, sl], ps, AF.Copy,
                                                 bias=0.0)
                    # V for this chunk's 4 s-tiles
                    for st in range(4 * c, 4 * (c + 1)):
                        ps = ps_w.tile([128, C], f32, name="v_ps", tag="wps")
                        for kt in range(CT):
                            nc.tensor.matmul(
                                ps,
                                hh[kt][:, 128 * st:128 * (st + 1)],
                                wv_sb[:, kt, :],
                                start=(kt == 0), stop=(kt == CT - 1))
                        nc.scalar.activation(
                            vaug[:, st // 2, :, st % 2, HS:128],
                            ps.rearrange("p (h e) -> p h e", h=H),
                            AF.Copy, bias=0.0)

            # ============ attention + per-chunk proj/LN2/MLP ============
            p_att = reps.enter_context(tc.tile_pool(name=f"p_att{_rep}",
                                                    bufs=2))
            p_late = reps.enter_context(tc.tile_pool(name=f"p_late{_rep}",
                                                     bufs=2))
            ps_s = reps.enter_context(
                tc.tile_pool(name=f"ps_s{_rep}", bufs=2, space="PSUM"))
            ps_o = reps.enter_context(
                tc.tile_pool(name=f"ps_o{_rep}", bufs=1, space="PSUM"))

            def att_pair(c, p, pre_group=None, post_group=None):
                sl = slice(512 * c, 512 * (c + 1))
                if True:
                    psA = ps_o.tile([128, 512], f32, name="psA", tag="psA")
                    psB = ps_o.tile([128, 512], f32, name="psB", tag="psB")
                    NSP = NST // 2
                    for stp in range(NSP):
                        if pre_group is not None and stp % 2 == 0:
                            pre_group(stp // 2)
                        if post_group is not None and stp % 2 == 1:
                            # tail pieces fire at group ENDS so the next
                            # chunk's score matmuls stay ahead of them in
                            # the PE FIFO (they feed the exp stream).
                            post_group(stp // 2)
                        # e2 layout [h, par, t]: fp8 exps for the two s-tiles
                        # of this pair, contiguous per (h, par).
                        e2 = p_att.tile([128, 2, 2, 512], f8, name="e2",
                                        tag="e2", bufs=3)
                        for par in range(2):
                            st = 2 * stp + par
                            psS = ps_s.tile([128, 1024], f32, name="psS",
                                            tag="psS")
                            ssl = slice(128 * st, 128 * (st + 1))
                            nc.tensor.matmul(psS[:, 0:512],
                                             k_sb[p][0:64, ssl],
                                             q_sb[p][0:64, sl],
                                             start=True, stop=True)
                            nc.tensor.matmul(psS[:, 512:1024],
                                             k_sb[p][64:128, ssl],
                                             q_sb[p][64:128, sl],
                                             start=True, stop=True)
                            nc.scalar.activation(e2[:, :, par, :], psS,
                                                 AF.Exp, bias=zcol)
                        nc.tensor.matmul(psA, vaug[:, stp, 2 * p, :, :],
                                         e2[:, 0, :, :],
                                         perf_mode=PM.DoubleRow,
                                         start=(stp == 0),
                                         stop=(stp == NSP - 1))
                        nc.tensor.matmul(psB, vaug[:, stp, 2 * p + 1, :, :],
                                         e2[:, 1, :, :],
                                         perf_mode=PM.DoubleRow,
                                         start=(stp == 0),
                                         stop=(stp == NSP - 1))
                    recA = p_att.tile([64, 512], f32, name="recA", tag="recA",
                                      bufs=1)
                    recB = p_att.tile([64, 512], f32, name="recB", tag="recB",
                                      bufs=1)
                    nc.vector.reciprocal_approx_fast(recA, psA[0:64, :])
                    nc.vector.reciprocal_approx_fast(recB, psB[0:64, :])
                    ot = p_att.tile([128, 512], bf, name=f"oT{p}",
                                    tag=f"oT{p}")
                    nc.vector.tensor_mul(ot[0:64, :], psA[64:128, :], recA)
                    nc.vector.tensor_mul(ot[64:128, :], psB[64:128, :], recB)
                    return ot

            def proj_piece(c, mch, oT, y1, y1b):
                """One mch slice of out-proj + residual."""
                sl = slice(512 * c, 512 * (c + 1))
                if True:
                    xbo = p_late.tile([128, 512], f32, name=f"xbo{mch}",
                                      tag=f"xbo{mch}")
                    nc.sync.dma_start(
                        xbo, d_xbo[128 * mch:128 * (mch + 1), sl])
                    ps = ps_w.tile([128, 512], f32, name="op_ps", tag="wps")
                    for p in range(NP):
                        nc.tensor.matmul(
                            ps, wo_sb[:, p, 128 * mch:128 * (mch + 1)],
                            oT[p], start=(p == 0), stop=(p == NP - 1))
                    t = p_late.tile([128, 512], f32, name=f"y1_{mch}",
                                    tag=f"y1_{mch}")
                    nc.vector.tensor_add(t, ps, xbo)
                    y1.append(t)
                    tb = p_late.tile([128, 512], bf, name=f"y1b{mch}",
                                     tag=f"y1b{mch}")
                    nc.vector.tensor_copy(tb, t)
                    y1b.append(tb)

            def ln2_piece(y1b):
                mu2 = ps_w.tile([128, 512], f32, name="mu2", tag="wps")
                for kt in range(CT):
                    nc.tensor.matmul(mu2, cones, y1b[kt],
                                     start=(kt == 0), stop=(kt == CT - 1))
                xc2 = []
                for kt in range(CT):
                    t = p_late.tile([128, 512], bf, name=f"xc2_{kt}",
                                    tag=f"xc2_{kt}")
                    nc.vector.tensor_sub(t, y1b[kt], mu2)
                    xc2.append(t)
                sq2 = []
                for kt in range(CT):
                    t = p_late.tile([128, 512], bf, name=f"sq2_{kt}",
                                    tag=f"sq2_{kt}", bufs=1)
                    nc.vector.tensor_mul(t, xc2[kt], xc2[kt])
                    sq2.append(t)
                var2 = ps_w.tile([128, 512], f32, name="var2", tag="wps")
                for kt in range(CT):
                    nc.tensor.matmul(var2, cones, sq2[kt],
                                     start=(kt == 0), stop=(kt == CT - 1))
                rr2 = rsqrt_dve(p_late, var2, "ln2_")
                h2 = []
                for kt in range(CT):
                    t = p_late.tile([128, 512], bf, name=f"h2_{kt}",
                                    tag=f"h2_{kt}")
                    nc.vector.tensor_mul(t, xc2[kt], rr2)
                    h2.append(t)
                return h2

            def w1_piece(h2, g, jts, lo=0, w=512, slots=None):
                """MLP up-proj + gelu (tanh form) for the given jt indices."""
                for jt in jts:
                    if slots is None:
                        psW = ps_w.tile([128, w], f32, name="m_ps",
                                        tag="wps")
                    else:
                        pool, tag = slots[jt % len(slots)]
                        psW = pool.tile([128, w], f32, name="m_ps", tag=tag)
                    for kt in range(CT):
                        nc.tensor.matmul(
                            psW, w1_sb[:, kt, 128 * jt:128 * (jt + 1)],
                            h2[kt][:, lo:lo + w],
                            start=(kt == 0), stop=(kt == CT - 1))
                    tth = p_late.tile([128, w], bf, name="tth", tag="tth")
                    nc.scalar.activation(tth, psW, AF.Tanh, bias=zcol,
                                         scale=GELU_B)
                    gt = p_late.tile([128, w], bf, name=f"g{jt}",
                                     tag=f"g{jt}", bufs=1)
                    nc.vector.scalar_tensor_tensor(gt, tth, 1.0, psW,
                                                   ALU.add, ALU.mult)
                    g.append(gt)

            def w2_piece(c, mch, y1, g, lo=0, w=512, slots=None):
                """One mch slice of MLP down-proj + residual + out DMA."""
                sl = slice(512 * c + lo, 512 * c + lo + w)
                if slots is None:
                    psF = ps_w.tile([128, w], f32, name="f_ps", tag="wps")
                else:
                    pool, tag = slots[mch % len(slots)]
                    psF = pool.tile([128, w], f32, name="f_ps", tag=tag)
                for kt in range(JT):
                    nc.tensor.matmul(
                        psF, w2_sb[:, kt, 128 * mch:128 * (mch + 1)],
                        g[kt], start=(kt == 0), stop=(kt == JT - 1))
                nc.vector.tensor_add(y1[mch][:, lo:lo + w], psF,
                                     y1[mch][:, lo:lo + w])
                nc.sync.dma_start(
                    d_out[128 * mch:128 * (mch + 1), sl],
                    y1[mch][:, lo:lo + w])

            # Fine-grained software-pipelined emission: tail PIECES of
            # chunk c-1 are injected between the 4-s-tile groups of chunk
            # c's attention pairs (proj/LN2 under pair 0, W1+gelu under
            # pair 1, W2 under pair 2), so every engine FIFO alternates
            # exp-feeding matmuls with small bites of MLP work.
            st8 = {}

            def mk_t1(c, oT):
                y1, y1b = [], []
                st8[c] = (y1, y1b)

                def fn(j):
                    if j < CT:
                        proj_piece(c, j, oT, y1, y1b)
                    else:
                        st8[c, "h2"] = ln2_piece(y1b)
                return fn

            def mk_t2(c):
                g = []
                st8[c, "g"] = g

                def fn(j):
                    w1_piece(st8[c, "h2"], g, range(3 * j, 3 * j + 3))
                return fn

            def mk_t3(c):
                def fn(j):
                    if j < CT:
                        w2_piece(c, j, st8[c][0], st8[c, "g"])
                return fn

            def mk_qk(p):
                def fn(j):
                    sl = slice(512 * j, 512 * (j + 1))
                    for (wsb, dst) in ((wk_sb, k_sb), (wq_sb, q_sb)):
                        ps = ps_w.tile([128, 512], f32, name="qk_ps",
                                       tag="wps")
                        for kt in range(CT):
                            nc.tensor.matmul(
                                ps, wsb[:, kt, 128 * p:128 * (p + 1)],
                                hh[kt][:, sl],
                                start=(kt == 0), stop=(kt == CT - 1))
                        nc.scalar.activation(dst[p][:, sl], ps, AF.Copy,
                                             bias=0.0)
                return fn

            ot0 = att_pair(0, 0, pre_group=ln1_chunk)
            if _rep == 0:
                nc.sync.dma_start(wo_sb, d_wo)
                nc.sync.dma_start(
                    w1_sb, d_w1.rearrange("(kt p) m -> p kt m", p=128))
                nc.sync.dma_start(
                    w2_sb, d_w2.rearrange("(kt p) m -> p kt m", p=128))
            oT_prev = [ot0] + [att_pair(0, p, pre_group=mk_qk(p))
                               for p in range(1, NP)]
            for c in range(1, NCH):
                ot0 = att_pair(c, 0, post_group=mk_t1(c - 1, oT_prev))
                ot1 = att_pair(c, 1, post_group=mk_t2(c - 1))
                ot2 = att_pair(c, 2, post_group=mk_t3(c - 1))
                oT_prev = [ot0, ot1, ot2]
            # Final chunk: attention is done, so the psA/psB banks are free
            # — cycle them into the W1/W2 PSUM rotation to deepen the gelu
            # pipeline (the tanh->STT->matmul cycle otherwise paces it).
            cl = NCH - 1
            y1, y1b = [], []
            for mch in range(CT):
                proj_piece(cl, mch, oT_prev, y1, y1b)
            h2 = ln2_piece(y1b)
            fslots = [(ps_w, "wps"), (ps_o, "psA"), (ps_w, "wps"),
                      (ps_o, "psB")]
            g = []
            w1_piece(h2, g, range(JT), slots=fslots)
            for mch in range(CT):
                w2_piece(cl, mch, y1, g, slots=fslots)

    nc.compile()
    return nc


def prep_inputs(x, ln1_w, ln2_w, Wq, Wk, Wv, Wo, bo, W1, W2):
    """Host-side preprocessing. Returns per-core in_maps (list of dicts)."""
    x = np.asarray(x, np.float32)
    ln1_w = np.asarray(ln1_w, np.float32)
    ln2_w = np.asarray(ln2_w, np.float32)
    scale = C ** (-0.5)
    wq = ((ln1_w[:, None, None] * np.asarray(Wq, np.float32).transpose(1, 0, 2))
          .reshape(C, C) * scale).astype(_BF)
    wk = (ln1_w[:, None, None] * np.asarray(Wk, np.float32).transpose(1, 0, 2)) \
        .reshape(C, C).astype(_BF)
    wv = (ln1_w[:, None, None] * np.asarray(Wv, np.float32).transpose(1, 0, 2)) \
        .reshape(C, C).astype(_BF)
    wo2 = np.asarray(Wo, np.float32).reshape(NP, 2, HS, C) \
        .transpose(1, 2, 0, 3).reshape(128, NP, C).astype(_BF)
    w1 = (ln2_w[:, None] * np.asarray(W1, np.float32)).astype(_BF)
    w2 = (0.5 * np.asarray(W2, np.float32)).astype(_BF)   # 0.5 of tanh-gelu
    bo_col = np.asarray(bo, np.float32).reshape(C, 1)
    cones = np.full((128, 128), 1.0 / C, np.float32).astype(_BF)

    in_maps = []
    for b in range(B):
        xT = np.ascontiguousarray(x[b].T)          # [C, T] fp32
        in_maps.append({
            "xb": xT.astype(_BF),
            "xbo": xT + bo_col,
            "wq": wq, "wk": wk, "wv": wv, "wo": wo2,
            "w1": w1, "w2": w2, "cones": cones,
        })
    return in_maps


def run(inputs, trace=False, repeat=1):
    """Build + run on 8 cores. Returns (output [B,T,C] fp32, results obj)."""
    from concourse.bass_utils import run_bass_kernel_spmd

    in_maps = prep_inputs(**inputs)
    nc = build_program(repeat=repeat)
    res = run_bass_kernel_spmd(nc, in_maps, core_ids=list(range(B)), trace=trace)
    out = np.stack([np.asarray(r["out"]).T for r in res.results])
    return np.ascontiguousarray(out.astype(np.float32)), res


def kernel(**inputs):
    return run(inputs, trace=False)[0]
